# revision 1
# baseline (speedup 1.0000x reference)
"""3-layer GAT + mean-pool + linear head on 8 Trainium2 NeuronCores.

Strategy (edge-major, dst-sharded, bf16 gather table):
  - Host: sort edges by dst, group dst nodes into 128-node blocks, deal blocks
    to the 8 cores so SPMD slot s is load-matched on every core. Node ids are
    relabeled (core, slot, pos). Within a slot, edges are grouped by the src
    table *quarter* (dma_gather indices are int16, so the 100352-row table is
    gathered as 4 sub-tables), then dst-sorted; tiles of 128 edges.
  - Per layer, a per-node table T = [h | h@a_s | h@a_d | pad] (bf16, 256-byte
    rows) is built sharded and AllGather'd. Each core then, per slot:
      * dma_gather of T rows by src (4 quarter gathers, batched over 4 slots)
      * dma_gather of T rows by dst from the LOCAL shard (a_dst expansion)
      * ex = exp(leakyrelu(a_src[src] + a_dst[dst] + c*attr))   (c = We@ae)
      * per 128-edge tile, one DVE tensor_scalar builds the weighted
        indicator lhsT[e,n] = (iota==dst_local[e])*ex[e]; PE matmuls
        accumulate PSUM [128 nodes, 65] = [sum ex*h | sum ex].
      * epilogue: x' = relu(S/(D+eps) + b), next table rows via PE transpose
        + matmul with the next layer's packed weights.
  - Graph mean-pool via PE matmuls with a host-built (1/cnt)-weighted graph
    indicator accumulated across all slots, then the linear head. The host
    sums the 8 per-core partial outputs (+blin).
"""

import sys

for _p in ("/opt/trn_rl_repo",):
    if _p not in sys.path:
        sys.path.insert(0, _p)

import numpy as np
import ml_dtypes

import concourse.bacc as bacc
import concourse.bass as bass
import concourse.tile as tile
from concourse import bass_utils, mybir

# Problem constants (hardcoded per spec)
N = 100_000
E = 1_600_000
G = 256
HID = 64
NEG_SLOPE = 0.2
EPS = 1e-16

NCORES = 8
P = 128            # partitions / edge-tile size / node-block size
RW = 128           # table row width (bf16) -> 256 bytes
C_AS = 64          # a_src column
C_AD = 65          # a_dst column
TCOLS = 66         # meaningful table columns [h | a_src | a_dst]
NQ = 4             # src sub-table quarters
GB = 4             # slots per gather batch

F32 = mybir.dt.float32
BF16 = mybir.dt.bfloat16
I16 = mybir.dt.int16
U32 = mybir.dt.uint32

BF = ml_dtypes.bfloat16


class Prep:
    pass


# ----------------------------------------------------------------------------
# Host-side graph preprocessing
# ----------------------------------------------------------------------------

def _wrap16(flat: np.ndarray) -> np.ndarray:
    """int16 stream -> [128, n/16] wrapped layout (k at [k%16, k//16], x8)."""
    n = flat.shape[0]
    assert n % 16 == 0
    w = flat.reshape(n // 16, 16).T          # [16, n/16]
    return np.tile(w, (8, 1))                # [128, n/16]


def preprocess(edge_index: np.ndarray, edge_attr: np.ndarray, batch: np.ndarray) -> Prep:
    pr = Prep()
    src = edge_index[0].astype(np.int64)
    dst = edge_index[1].astype(np.int64)
    attr = edge_attr[:, 0].astype(np.float32)

    nblk = (N + P - 1) // P
    nblk_pad = ((nblk + NCORES - 1) // NCORES) * NCORES
    n_slots = nblk_pad // NCORES
    n_loc = n_slots * P
    n_tab = NCORES * n_loc
    qrows = n_tab // NQ
    assert qrows <= 32768 and n_tab % NQ == 0

    order = np.argsort(dst, kind="stable")
    dst_s = dst[order]
    src_s = src[order]
    attr_s = attr[order]
    blk_of_edge = dst_s // P
    cnt = np.bincount(blk_of_edge, minlength=nblk_pad)
    seg = np.zeros(nblk_pad + 1, np.int64)
    seg[1:] = np.cumsum(cnt)

    ranked = np.argsort(-cnt, kind="stable")
    block_of = ranked.reshape(n_slots, NCORES)   # [slot, core] -> block

    # node relabel
    new_core = np.full(N, -1, np.int32)
    new_loc = np.full(N, -1, np.int32)
    for s in range(n_slots):
        for c in range(NCORES):
            b = block_of[s, c]
            lo, hi = b * P, min(b * P + P, N)
            if hi <= lo:
                continue
            ids = np.arange(lo, hi)
            new_core[ids] = c
            new_loc[ids] = s * P + (ids - lo)
    assert (new_core >= 0).all()
    new_glob = new_core.astype(np.int64) * n_loc + new_loc.astype(np.int64)

    # per (core, slot, quarter): edge lists (dst-sorted within)
    # and uniform-across-cores tile counts ntq[s][q]
    edges_csq = {}
    cnt_csq = np.zeros((NCORES, n_slots, NQ), np.int64)
    for s in range(n_slots):
        for c in range(NCORES):
            b = block_of[s, c]
            e0, e1 = seg[b], seg[b + 1]
            if e1 <= e0:
                for q in range(NQ):
                    edges_csq[(c, s, q)] = np.empty(0, np.int64)
                continue
            ee = np.arange(e0, e1)
            qq = new_glob[src_s[ee]] // qrows
            for q in range(NQ):
                sel = ee[qq == q]
                edges_csq[(c, s, q)] = sel
                cnt_csq[c, s, q] = sel.shape[0]

    ntq = np.maximum(0, (cnt_csq.max(axis=0) + P - 1) // P)   # [slot, q]
    # ensure every slot has at least one tile
    for s in range(n_slots):
        if ntq[s].sum() == 0:
            ntq[s, 0] = 1
    tb = ntq.sum(axis=1).astype(np.int64)                     # tiles per slot
    tile_off = np.zeros(n_slots + 1, np.int64)
    tile_off[1:] = np.cumsum(tb)
    TT = int(tile_off[-1])

    # per-slot per-tile metadata in slot-tile order (q-major)
    dstl = np.full((NCORES, P, TT), -1.0, np.float32)
    attr_m = np.zeros((NCORES, P, TT), np.float32)
    # int16 index streams
    src16 = np.zeros((NCORES, n_slots, NQ), object)
    dst16 = np.zeros((NCORES, n_slots), object)

    for s in range(n_slots):
        for c in range(NCORES):
            jbase = 0
            dflat = np.zeros(int(tb[s]) * P, np.int16)
            for q in range(NQ):
                nt = int(ntq[s, q])
                if nt == 0:
                    continue
                ee = edges_csq[(c, s, q)]
                k = ee.shape[0]
                sl = np.zeros(nt * P, np.int16)
                if k:
                    sl[:k] = (new_glob[src_s[ee]] - q * qrows).astype(np.int16)
                    pp = np.arange(k) % P
                    jj = np.arange(k) // P + tile_off[s] + jbase
                    dl = (dst_s[ee] - block_of[s, c] * P).astype(np.float32)
                    dstl[c, pp, jj] = dl
                    attr_m[c, pp, jj] = attr_s[ee]
                    dflat[jbase * P: jbase * P + k] = (s * P + dl).astype(np.int16)
                src16[c, s, q] = sl
                jbase += nt
            dst16[c, s] = dflat

    # gather-batch index blobs
    n_gb = (n_slots + GB - 1) // GB
    gb_slots = [list(range(g * GB, min((g + 1) * GB, n_slots))) for g in range(n_gb)]
    # src idx columns per (gb, q); dst idx columns per gb
    sidx_cols = np.zeros((n_gb, NQ), np.int64)   # lengths/16
    didx_cols = np.zeros(n_gb, np.int64)
    for g, sl in enumerate(gb_slots):
        for q in range(NQ):
            sidx_cols[g, q] = sum(int(ntq[s, q]) * P for s in sl) // 16
        didx_cols[g] = sum(int(tb[s]) * P for s in sl) // 16
    sidx_off = np.zeros((n_gb, NQ + 1), np.int64)
    soff = 0
    for g in range(n_gb):
        for q in range(NQ):
            sidx_off[g, q] = soff
            soff += sidx_cols[g, q]
        sidx_off[g, NQ] = soff
    didx_off = np.zeros(n_gb + 1, np.int64)
    didx_off[1:] = np.cumsum(didx_cols)
    SC, DC = int(soff), int(didx_off[-1])

    sidx = np.zeros((NCORES, P, SC), np.int16)
    didx = np.zeros((NCORES, P, DC), np.int16)
    for c in range(NCORES):
        for g, sl in enumerate(gb_slots):
            for q in range(NQ):
                parts = [src16[c, s, q] for s in sl if int(ntq[s, q])]
                if not parts:
                    continue
                flat = np.concatenate(parts)
                o = int(sidx_off[g, q])
                sidx[c, :, o:o + flat.shape[0] // 16] = _wrap16(flat)
            parts = [dst16[c, s] for s in sl]
            flat = np.concatenate(parts)
            o = int(didx_off[g])
            didx[c, :, o:o + flat.shape[0] // 16] = _wrap16(flat)

    # meta: [dstl | attr] as f32 blob per slot (dstl also as bf16 is derived
    # on device-side scalar ops; we ship dstl bf16 + attr f32 separately)
    dstl16 = dstl
    attr32 = attr_m

    # graph pooling: per-node graph id and 1/cnt weight, laid out [128, n_slots]
    cnt_g = np.bincount(batch.astype(np.int64), minlength=G).astype(np.float32)
    wg = 1.0 / np.maximum(cnt_g, 1.0)
    g_of = batch.astype(np.int64)
    gid = np.full((NCORES, n_loc), -1.0, np.float32)
    winv = np.zeros((NCORES, n_loc), np.float32)
    ids = np.arange(N)
    gid[new_core[ids], new_loc[ids]] = g_of.astype(np.float32)
    winv[new_core[ids], new_loc[ids]] = wg[g_of]
    # [n_loc] -> [128, n_slots] (node s*128+p at [p, s])
    gid = gid.reshape(NCORES, n_slots, P).transpose(0, 2, 1)
    winv = winv.reshape(NCORES, n_slots, P).transpose(0, 2, 1)

    pr.n_slots, pr.n_loc, pr.n_tab, pr.TT, pr.qrows = n_slots, n_loc, n_tab, TT, qrows
    pr.ntq, pr.tb, pr.tile_off = ntq, tb, tile_off
    pr.n_gb, pr.gb_slots = n_gb, gb_slots
    pr.sidx_cols, pr.sidx_off, pr.didx_cols, pr.didx_off = \
        sidx_cols, sidx_off, didx_cols, didx_off
    pr.SC, pr.DC = SC, DC
    pr.sidx, pr.didx = sidx, didx
    pr.dstl16, pr.attr32 = dstl16, attr32
    pr.gid, pr.winv = gid, winv
    pr.new_core, pr.new_loc = new_core, new_loc
    return pr


def pack_weights(inputs: dict) -> dict:
    w = {}
    for l in (1, 2, 3):
        W = np.asarray(inputs[f"W{l}"], np.float32)
        a_s = np.asarray(inputs[f"as{l}"], np.float32)
        a_d = np.asarray(inputs[f"ad{l}"], np.float32)
        # x @ wext = [h | h@a_s | h@a_d]
        w[f"wext{l}"] = np.concatenate(
            [W, (W @ a_s)[:, None], (W @ a_d)[:, None]], axis=1)
        w[f"c{l}"] = float(np.asarray(inputs[f"We{l}"], np.float32)[0]
                           @ np.asarray(inputs[f"ae{l}"], np.float32))
        w[f"brep{l}"] = np.tile(np.asarray(inputs[f"b{l}"], np.float32)[None, :], (P, 1))
    w["wlin"] = np.asarray(inputs["Wlin"], np.float32)
    w["blin"] = float(np.asarray(inputs["blin"], np.float32)[0])
    return w


# ----------------------------------------------------------------------------
# Device program
# ----------------------------------------------------------------------------

def build_program(pr: Prep, c_scal, use_act_lrelu: bool = True):
    n_slots, n_loc, n_tab, TT = pr.n_slots, pr.n_loc, pr.n_tab, pr.TT
    ntq, tb, tile_off = pr.ntq, pr.tb, pr.tile_off

    nc = bacc.Bacc("TRN2", target_bir_lowering=False, debug=False,
                   num_devices=NCORES)
    rg = [list(range(NCORES))]

    meta_d = nc.dram_tensor("dstl", [P, TT], F32, kind="ExternalInput")
    attr_d = nc.dram_tensor("attr", [P, TT], F32, kind="ExternalInput")
    sidx_d = nc.dram_tensor("sidx", [P, pr.SC], I16, kind="ExternalInput")
    didx_d = nc.dram_tensor("didx", [P, pr.DC], I16, kind="ExternalInput")
    xT_d = nc.dram_tensor("xT", [HID, n_loc], F32, kind="ExternalInput")
    gid_d = nc.dram_tensor("gid", [P, n_slots], F32, kind="ExternalInput")
    winv_d = nc.dram_tensor("winv", [P, n_slots], F32, kind="ExternalInput")
    wext_d = [nc.dram_tensor(f"wext{l}", [HID, TCOLS], F32, kind="ExternalInput")
              for l in (1, 2, 3)]
    brep_d = [nc.dram_tensor(f"brep{l}", [P, HID], F32, kind="ExternalInput")
              for l in (1, 2, 3)]
    wlin_d = nc.dram_tensor("wlin", [HID, 1], F32, kind="ExternalInput")
    iota_d = nc.dram_tensor("iota", [P, P], BF16, kind="ExternalInput")
    iotg_d = nc.dram_tensor("iotg", [P, G], F32, kind="ExternalInput")
    ident_d = nc.dram_tensor("ident", [P, P], F32, kind="ExternalInput")
    out_d = nc.dram_tensor("out", [P, G // P], F32, kind="ExternalOutput")

    T_full = [nc.dram_tensor(f"T{l}", [n_tab, RW], BF16, kind="Internal",
                             addr_space="Shared") for l in (1, 2, 3)]
    T_sh = [nc.dram_tensor(f"Tsh{l}", [n_loc, RW], BF16, kind="Internal")
            for l in (1, 2, 3)]

    # chunk layout tables (python ints, compile-time)
    chs_src = {}   # (gb, q, s) -> start chunk of that run in hs_src
    chd = {}       # (gb, s) -> start chunk in hs_dst
    ch_src_tot = {}
    ch_dst_tot = {}
    for g, sl in enumerate(pr.gb_slots):
        o = 0
        for q in range(NQ):
            for s in sl:
                chs_src[(g, q, s)] = o
                o += int(ntq[s, q])
        ch_src_tot[g] = o
        o = 0
        for s in sl:
            chd[(g, s)] = o
            o += int(tb[s])
        ch_dst_tot[g] = o
    max_src_ch = max(ch_src_tot.values())
    max_dst_ch = max(ch_dst_tot.values())

    with tile.TileContext(nc) as tc:
        with (
            tc.tile_pool(name="const", bufs=1) as cpool,
            tc.tile_pool(name="sbuf", bufs=4) as spool,
            tc.tile_pool(name="gath", bufs=3) as gpool,
            tc.tile_pool(name="psum", bufs=2, space="PSUM") as ppool,
            tc.tile_pool(name="psum1", bufs=1, space="PSUM") as ppoolA,
            tc.tile_pool(name="ppool2", bufs=1, space="PSUM") as ppool1,
        ):
            iota_sb = cpool.tile([P, P], BF16, tag="iota")
            nc.sync.dma_start(out=iota_sb[:], in_=iota_d[:, :])
            ident_sb = cpool.tile([P, P], F32, tag="ident")
            nc.sync.dma_start(out=ident_sb[:], in_=ident_d[:, :])
            wext_sb, brep_sb = [], []
            for l in range(3):
                t1 = cpool.tile([HID, TCOLS], F32, tag=f"wext{l}", name=f"wext{l}")
                nc.sync.dma_start(out=t1[:], in_=wext_d[l][:, :])
                wext_sb.append(t1)
                t2 = cpool.tile([P, HID], F32, tag=f"brep{l}", name=f"brep{l}")
                nc.sync.dma_start(out=t2[:], in_=brep_d[l][:, :])
                brep_sb.append(t2)
            wlin_sb = cpool.tile([HID, 1], F32, tag="wlin")
            nc.sync.dma_start(out=wlin_sb[:], in_=wlin_d[:, :])
            xT_sb = cpool.tile([HID, n_loc], F32, tag="xT")
            nc.sync.dma_start(out=xT_sb[:], in_=xT_d[:, :])
            gid_sb = cpool.tile([P, n_slots], F32, tag="gid")
            nc.sync.dma_start(out=gid_sb[:], in_=gid_d[:, :])
            winv_sb = cpool.tile([P, n_slots], F32, tag="winv")
            nc.sync.dma_start(out=winv_sb[:], in_=winv_d[:, :])
            iotg_sb = cpool.tile([P, G], F32, tag="iotg")
            nc.sync.dma_start(out=iotg_sb[:], in_=iotg_d[:, :])

            # ---- layer-1 table shard
            for s in range(n_slots):
                t_ps = ppoolA.tile([P, TCOLS], F32, tag="tps")
                nc.tensor.matmul(out=t_ps[:], lhsT=xT_sb[:, s * P:(s + 1) * P],
                                 rhs=wext_sb[0][:], start=True, stop=True)
                trow = spool.tile([P, RW], BF16, tag="trow")
                nc.scalar.copy(out=trow[:, 0:TCOLS], in_=t_ps[:])
                nc.vector.memset(trow[:, TCOLS:RW], 0.0)
                nc.sync.dma_start(out=T_sh[0][s * P:(s + 1) * P, :],
                                  in_=trow[:])

            nc.gpsimd.collective_compute(
                "AllGather", mybir.AluOpType.bypass, replica_groups=rg,
                ins=[T_sh[0].ap().opt()], outs=[T_full[0].ap().opt()])

            pool_ps = [ppool1.tile([P, HID], F32, tag=f"pool{h}", name=f"pool{h}")
                       for h in range(G // P)]

            lrelu = mybir.ActivationFunctionType.Lrelu if use_act_lrelu else None

            for l in range(3):
                last = l == 2
                for g, sl in enumerate(pr.gb_slots):
                    # ---- gathers for this batch of slots
                    hs_src = gpool.tile([P, max_src_ch * RW], BF16, tag="hsrc")
                    hs_dst = gpool.tile([P, max_dst_ch * RW], BF16, tag="hdst")
                    for q in range(NQ):
                        ncols = int(pr.sidx_cols[g, q])
                        if ncols == 0:
                            continue
                        o = int(pr.sidx_off[g, q])
                        idx_sb = spool.tile([P, ncols], I16, tag="sidx",
                                            name=f"sidx_{l}_{g}_{q}")
                        nc.sync.dma_start(out=idx_sb[:],
                                          in_=sidx_d[:, o:o + ncols])
                        nidx = ncols * 16
                        c0 = chs_src[(g, q, sl[0])]
                        nch = nidx // P
                        # split into <=1024-index pieces (SWDGE ring limit)
                        npieces = (nch + 7) // 8
                        for pi in range(npieces):
                            ca = pi * ((nch + npieces - 1) // npieces)
                            cb = min(nch, (pi + 1) * ((nch + npieces - 1) // npieces))
                            if cb <= ca:
                                continue
                            nc.gpsimd.dma_gather(
                                out_ap=hs_src[:, (c0 + ca) * RW:(c0 + cb) * RW]
                                .rearrange("p (t c) -> p t c", c=RW),
                                in_ap=T_full[l][q * pr.qrows:(q + 1) * pr.qrows, :],
                                idxs_ap=idx_sb[:, ca * 8:cb * 8],
                                num_idxs=(cb - ca) * P,
                                num_idxs_reg=(cb - ca) * P, elem_size=RW)
                    ncols = int(pr.didx_cols[g])
                    o = int(pr.didx_off[g])
                    didx_sb = spool.tile([P, ncols], I16, tag="didx",
                                         name=f"didx_{l}_{g}")
                    nc.sync.dma_start(out=didx_sb[:], in_=didx_d[:, o:o + ncols])
                    nidx = ncols * 16
                    nch = nidx // P
                    npieces = (nch + 7) // 8
                    for pi in range(npieces):
                        ca = pi * ((nch + npieces - 1) // npieces)
                        cb = min(nch, (pi + 1) * ((nch + npieces - 1) // npieces))
                        if cb <= ca:
                            continue
                        nc.gpsimd.dma_gather(
                            out_ap=hs_dst[:, ca * RW:cb * RW]
                            .rearrange("p (t c) -> p t c", c=RW),
                            in_ap=T_sh[l][:, :],
                            idxs_ap=didx_sb[:, ca * 8:cb * 8],
                            num_idxs=(cb - ca) * P,
                            num_idxs_reg=(cb - ca) * P, elem_size=RW)

                    hs3 = hs_src[:].rearrange("p (t c) -> p t c", c=RW)
                    hd3 = hs_dst[:].rearrange("p (t c) -> p t c", c=RW)

                    for s in sl:
                        t = int(tb[s])
                        o = int(tile_off[s])
                        dstl_sb = spool.tile([P, t], F32, tag="dstl",
                                             name=f"dstl_{l}_{s}")
                        nc.sync.dma_start(out=dstl_sb[:], in_=meta_d[:, o:o + t])
                        attr_sb = spool.tile([P, t], F32, tag="attrm",
                                             name=f"attr_{l}_{s}")
                        nc.sync.dma_start(out=attr_sb[:], in_=attr_d[:, o:o + t])

                        # X = a_src[src] + a_dst[dst]  (bf16+bf16 -> f32)
                        X = spool.tile([P, t], F32, tag="xsum", name=f"X_{l}_{s}")
                        dj = chd[(g, s)]
                        adst_v = hd3[:, dj:dj + t, C_AD:C_AD + 1] \
                            .rearrange("p t c -> p (t c)")
                        jb = 0
                        for q in range(NQ):
                            nt = int(ntq[s, q])
                            if nt == 0:
                                continue
                            cj = chs_src[(g, q, s)]
                            asrc_v = hs3[:, cj:cj + nt, C_AS:C_AS + 1] \
                                .rearrange("p t c -> p (t c)")
                            nc.vector.tensor_tensor(
                                out=X[:, jb:jb + nt], in0=asrc_v,
                                in1=adst_v[:, jb:jb + nt],
                                op=mybir.AluOpType.add)
                            jb += nt
                        s2 = spool.tile([P, t], F32, tag="s2", name=f"s2_{l}_{s}")
                        nc.vector.scalar_tensor_tensor(
                            out=s2[:], in0=attr_sb[:], scalar=float(c_scal[l]),
                            in1=X[:], op0=mybir.AluOpType.mult,
                            op1=mybir.AluOpType.add)
                        alf = spool.tile([P, t], F32, tag="alf", name=f"alf_{l}_{s}")
                        if use_act_lrelu:
                            nc.scalar.activation(out=alf[:], in_=s2[:],
                                                 func=lrelu, alpha=NEG_SLOPE)
                        else:
                            nc.vector.scalar_tensor_tensor(
                                out=alf[:], in0=s2[:], scalar=NEG_SLOPE,
                                in1=s2[:], op0=mybir.AluOpType.mult,
                                op1=mybir.AluOpType.max)
                        ex = spool.tile([P, t], F32, tag="ex", name=f"ex_{l}_{s}")
                        nc.scalar.activation(out=ex[:], in_=alf[:],
                                             func=mybir.ActivationFunctionType.Exp)

                        # ones into the a_src column of this slot's chunks
                        jb = 0
                        agg = ppool.tile([P, C_AS + 1], F32, tag="agg",
                                         name=f"agg_{l}_{s}")
                        first = True
                        for q in range(NQ):
                            nt = int(ntq[s, q])
                            if nt == 0:
                                continue
                            cj = chs_src[(g, q, s)]
                            nc.vector.memset(
                                hs3[:, cj:cj + nt, C_AS:C_AS + 1], 1.0)
                            for k in range(nt):
                                j = jb + k
                                indw = spool.tile([P, P], BF16, tag="indw",
                                                  name=f"iw_{l}_{s}_{j}")
                                nc.vector.tensor_scalar(
                                    out=indw[:], in0=iota_sb[:],
                                    scalar1=dstl_sb[:, j:j + 1],
                                    scalar2=ex[:, j:j + 1],
                                    op0=mybir.AluOpType.is_equal,
                                    op1=mybir.AluOpType.mult)
                                nc.tensor.matmul(
                                    out=agg[:], lhsT=indw[:],
                                    rhs=hs3[:, cj + k, 0:C_AS + 1],
                                    start=first, stop=(j == t - 1),
                                    skip_group_check=True)
                                first = False
                            jb += nt

                        # epilogue
                        dpe = spool.tile([P, 1], F32, tag="dpe", name=f"dpe_{l}_{s}")
                        nc.vector.tensor_scalar_add(
                            out=dpe[:], in0=agg[:, C_AS:C_AS + 1], scalar1=EPS)
                        rcp = spool.tile([P, 1], F32, tag="rcp", name=f"rcp_{l}_{s}")
                        nc.vector.reciprocal(out=rcp[:], in_=dpe[:])
                        x2 = spool.tile([P, HID], F32, tag="x2", name=f"x2_{l}_{s}")
                        nc.scalar.activation(
                            out=x2[:], in_=agg[:, 0:C_AS],
                            func=mybir.ActivationFunctionType.Copy,
                            scale=rcp[:, 0:1])
                        x2b = spool.tile([P, HID], F32, tag="x2b", name=f"x2b_{l}_{s}")
                        nc.vector.tensor_tensor(out=x2b[:], in0=x2[:],
                                                in1=brep_sb[l][:],
                                                op=mybir.AluOpType.add)
                        if not last:
                            x3 = spool.tile([P, HID], F32, tag="x3",
                                            name=f"x3_{l}_{s}")
                            nc.scalar.activation(
                                out=x3[:], in_=x2b[:],
                                func=mybir.ActivationFunctionType.Relu)
                            xt_ps = ppoolA.tile([HID, P], F32, tag="xtps")
                            nc.tensor.transpose(out=xt_ps[:], in_=x3[:],
                                                identity=ident_sb[:])
                            xt_sb = spool.tile([HID, P], F32, tag="xtsb",
                                               name=f"xt_{l}_{s}")
                            nc.scalar.copy(out=xt_sb[:], in_=xt_ps[:])
                            tn_ps = ppoolA.tile([P, TCOLS], F32, tag="tps")
                            nc.tensor.matmul(out=tn_ps[:], lhsT=xt_sb[:],
                                             rhs=wext_sb[l + 1][:],
                                             start=True, stop=True)
                            trow = spool.tile([P, RW], BF16, tag="trow",
                                              name=f"trow_{l}_{s}")
                            nc.scalar.copy(out=trow[:, 0:TCOLS], in_=tn_ps[:])
                            nc.vector.memset(trow[:, TCOLS:RW], 0.0)
                            nc.sync.dma_start(
                                out=T_sh[l + 1][s * P:(s + 1) * P, :],
                                in_=trow[:])
                        else:
                            for h in range(G // P):
                                gih = spool.tile([P, P], F32, tag="gih",
                                                 name=f"gi_{s}_{h}")
                                nc.vector.tensor_scalar(
                                    out=gih[:],
                                    in0=iotg_sb[:, h * P:(h + 1) * P],
                                    scalar1=gid_sb[:, s:s + 1],
                                    scalar2=winv_sb[:, s:s + 1],
                                    op0=mybir.AluOpType.is_equal,
                                    op1=mybir.AluOpType.mult)
                                nc.tensor.matmul(
                                    out=pool_ps[h][:], lhsT=gih[:], rhs=x2b[:],
                                    start=(s == 0), stop=(s == n_slots - 1),
                                    skip_group_check=True)

                if not last:
                    nc.gpsimd.collective_compute(
                        "AllGather", mybir.AluOpType.bypass, replica_groups=rg,
                        ins=[T_sh[l + 1].ap().opt()],
                        outs=[T_full[l + 1].ap().opt()])

            # ---- head
            out_sb = spool.tile([P, G // P], F32, tag="outsb")
            for h in range(G // P):
                pool_sb = spool.tile([P, HID], F32, tag="poolsb",
                                     name=f"poolsb{h}")
                nc.vector.tensor_copy(out=pool_sb[:], in_=pool_ps[h][:])
                pt_ps = ppoolA.tile([HID, P], F32, tag="xtps")
                nc.tensor.transpose(out=pt_ps[:], in_=pool_sb[:],
                                    identity=ident_sb[:])
                pt_sb = spool.tile([HID, P], F32, tag="xtsb", name=f"ptsb{h}")
                nc.scalar.copy(out=pt_sb[:], in_=pt_ps[:])
                o_ps = ppoolA.tile([P, 1], F32, tag="tps", name=f"o_ps{h}")
                nc.tensor.matmul(out=o_ps[:], lhsT=pt_sb[:], rhs=wlin_sb[:],
                                 start=True, stop=True)
                nc.vector.tensor_copy(out=out_sb[:, h:h + 1], in_=o_ps[:])
            nc.sync.dma_start(out=out_d[:, :], in_=out_sb[:])

    nc.compile()
    return nc


# ----------------------------------------------------------------------------
# Entry point
# ----------------------------------------------------------------------------

def make_inmaps(pr: Prep, w: dict, x: np.ndarray):
    iota = np.tile(np.arange(P, dtype=np.float32)[None, :], (P, 1)).astype(BF)
    iotg = np.tile(np.arange(G, dtype=np.float32)[None, :], (P, 1))
    ident = np.eye(P, dtype=np.float32)
    in_maps = []
    for c in range(NCORES):
        xT_c = np.zeros((HID, pr.n_loc), np.float32)
        mask = pr.new_core == c
        xT_c[:, pr.new_loc[mask]] = x[mask].T
        m = {
            "dstl": pr.dstl16[c],
            "attr": pr.attr32[c],
            "sidx": pr.sidx[c],
            "didx": pr.didx[c],
            "xT": xT_c,
            "gid": pr.gid[c],
            "winv": pr.winv[c],
            "wlin": w["wlin"],
            "iota": iota,
            "iotg": iotg,
            "ident": ident,
        }
        for l in (1, 2, 3):
            m[f"wext{l}"] = w[f"wext{l}"]
            m[f"brep{l}"] = w[f"brep{l}"]
        in_maps.append(m)
    return in_maps


def kernel(**inputs) -> np.ndarray:
    inputs = {k: np.asarray(v) for k, v in inputs.items()}
    pr = preprocess(inputs["edge_index"], inputs["edge_attr"], inputs["batch"])
    w = pack_weights(inputs)
    nc = build_program(pr, [w["c1"], w["c2"], w["c3"]], use_act_lrelu=False)
    in_maps = make_inmaps(pr, w, np.asarray(inputs["x"], np.float32))
    res = bass_utils.run_bass_kernel_spmd(nc, in_maps,
                                          core_ids=list(range(NCORES)))
    out = np.zeros(G, np.float64)
    for c in range(NCORES):
        oc = res.results[c]["out"]
        out += oc.T.reshape(-1).astype(np.float64)
    return (out + w["blin"]).astype(np.float32)



# revision 9
# speedup vs baseline: 1.5058x; 1.5058x over previous
"""3-layer GAT + mean-pool + linear head on 8 Trainium2 NeuronCores.

V2 strategy (dst-partition-aligned edge layout, no dst gather):
  - Host: nodes dealt to 8 cores by in-degree round-robin, then a vectorized
    local-search pass swaps equal-degree nodes between cores to even out each
    destination's in-edge spread across the 4 source table quarters (quarter =
    core pair, forced by the AllGather row order and int16 gather indices).
  - Within a core, nodes sort by in-degree into 128-node slots; an edge lives
    at partition p = its dst's slot position, column j = its rank among that
    (dst, quarter)'s edges.  a_dst therefore becomes a per-partition scalar
    (adcol), eliminating the per-edge dst gather entirely.
  - Per layer a table T[node] = [h | 1 | h@a_s | h@a_d | pad] (bf16, 256B
    rows) is the gather source.  Layer 1's table is precomputed on the host
    and shipped as an input (no AllGather).  Layers 2/3 build their shard
    during the previous layer's epilogue and AllGather it.
  - Per slot: ex = exp(leakyrelu(asrc[src] + adcol[p] + c*attr)) (bf16); per
    edge-column the gathered row [h|1] is scaled by ex (DVE, 4x bf16 mode)
    and accumulated into PSUM [128, 65] by a matmul against a constant
    identity (lhsT), yielding [sum ex*h | sum ex] per dst node.
  - Epilogue: x' = x2 + b with x2 = num/(den+eps); next-layer table rows via
    PE transpose + matmul with packed weights; graph mean-pool via PE with a
    host-built (1/cnt)-weighted graph indicator; host sums per-core partials.
"""

import sys

for _p in ("/opt/trn_rl_repo",):
    if _p not in sys.path:
        sys.path.insert(0, _p)

import numpy as np
import ml_dtypes

import concourse.bacc as bacc
import concourse.bass as bass
import concourse.tile as tile
from concourse import bass_utils, mybir

# Problem constants (hardcoded per spec)
N = 100_000
E = 1_600_000
G = 256
HID = 64
NEG_SLOPE = 0.2
EPS = 1e-16

NCORES = 8
NQ = 4             # src table quarters (int16 gather index limit)
P = 128            # partitions / node-block size
RW = 128           # table row width (bf16 cols) -> 256 bytes
C_ONE = 64         # constant-1 column (denominator rides the matmul)
C_AS = 65          # a_src column
C_AD = 66          # a_dst column
TCOLS = 67         # meaningful table columns
PAD_NEG = -1.0e30  # attr_eff value on pad edges -> ex = exp(-inf) = 0

GB_COLS = 96       # target gathered columns per gather batch
GB_SLOTS = 8       # max slots per gather batch
PIECE = 1024       # max indices per dma_gather instruction (HW SWDGE ring limit)

F32 = mybir.dt.float32
BF16 = mybir.dt.bfloat16
I16 = mybir.dt.int16

BF = ml_dtypes.bfloat16

N_SWEEPS = 12      # quarter-balance local search sweeps


class Prep:
    pass


# ----------------------------------------------------------------------------
# Host-side graph preprocessing
# ----------------------------------------------------------------------------

def _wrap16(flat: np.ndarray) -> np.ndarray:
    """int16 stream -> [128, n/16] wrapped layout (k at [k%16, k//16], x8)."""
    n = flat.shape[0]
    assert n % 16 == 0
    w = flat.reshape(n // 16, 16).T          # [16, n/16]
    return np.tile(w, (8, 1))                # [128, n/16]


def _balance_quarters(src, dst, deg_in):
    """Assign nodes to cores (equal counts, aligned degree profiles) while
    evening out each dst's in-edge spread across the 4 quarters.  Batched
    local search with live counts + re-verification.  Returns (core_of, C)
    where C[n, q] = in-edges of n from quarter q."""
    ceil4 = (deg_in + 3) // 4
    nodes = np.arange(N)
    order = np.argsort(-deg_in, kind="stable")
    core_of = np.empty(N, np.int32)
    core_of[order] = np.arange(N) % NCORES

    eorder = np.argsort(src, kind="stable")
    estarts = np.searchsorted(src[eorder], np.arange(N + 1))
    dst_sorted = dst[eorder]

    C = np.zeros((N, NQ), np.int32)
    np.add.at(C, (dst, core_of[src] // 2), 1)
    for _sw in range(N_SWEEPS):
        q_of = (core_of // 2).astype(np.int64)
        addpen = (C >= ceil4[:, None])
        decgain = (C > ceil4[:, None])
        A = np.zeros((N, NQ), np.int64)
        for q in range(NQ):
            np.add.at(A[:, q], src, addpen[dst, q])
        B = np.zeros(N, np.int64)
        np.add.at(B, src, decgain[dst, q_of[src]])
        A2 = A.copy()
        A2[nodes, q_of] = 10**9
        best = np.argmin(A2, axis=1).astype(np.int64)
        gain = B - A2[nodes, best]
        movers = nodes[gain > 0]
        if len(movers) == 0:
            break
        movers = movers[np.argsort(-gain[movers], kind="stable")]
        applied = 0
        for a0 in range(0, len(movers), 2048):
            ch = movers[a0:a0 + 2048]
            eds = [dst_sorted[estarts[s]:estarts[s + 1]] for s in ch]
            lens = np.array([len(e) for e in eds])
            if lens.sum() == 0:
                continue
            flat = np.concatenate(eds)
            seg = np.repeat(np.arange(len(ch)), lens)
            qcur = (core_of[ch] // 2).astype(np.int64)
            Bv = np.zeros(len(ch), np.int64)
            np.add.at(Bv, seg, (C[flat, qcur[seg]] > ceil4[flat]))
            Av = np.zeros((len(ch), NQ), np.int64)
            for q in range(NQ):
                np.add.at(Av[:, q], seg, (C[flat, q] >= ceil4[flat]))
            Av[np.arange(len(ch)), qcur] = 10**9
            bq = np.argmin(Av, axis=1)
            gv = Bv - Av[np.arange(len(ch)), bq]
            ok = gv > 0
            if not ok.any():
                continue
            ids = ch[ok]
            icur = qcur[ok]
            ibest = bq[ok]
            ideg = deg_in[ids]
            lo = np.minimum(icur, ibest)
            hi = np.maximum(icur, ibest)
            fwd = icur == lo
            key = (ideg * 16 + lo * 4 + hi) * 2 + fwd
            srt = np.argsort(key, kind="stable")
            ids, key, fwd = ids[srt], key[srt], fwd[srt]
            uk, st2 = np.unique(key >> 1, return_index=True)
            st2 = list(st2) + [len(key)]
            swap_a, swap_b = [], []
            for gi in range(len(uk)):
                b0, b1 = st2[gi], st2[gi + 1]
                ids_f = ids[b0:b1][fwd[b0:b1]]
                ids_b = ids[b0:b1][~fwd[b0:b1]]
                m = min(len(ids_f), len(ids_b))
                if m:
                    swap_a.append(ids_f[:m])
                    swap_b.append(ids_b[:m])
            if not swap_a:
                continue
            sa = np.concatenate(swap_a)
            sb = np.concatenate(swap_b)
            oq_a = (core_of[sa] // 2).astype(np.int64)
            oq_b = (core_of[sb] // 2).astype(np.int64)
            tmp = core_of[sa].copy()
            core_of[sa] = core_of[sb]
            core_of[sb] = tmp
            moved = np.concatenate([sa, sb])
            oldq = np.concatenate([oq_a, oq_b])
            newq = np.concatenate([oq_b, oq_a])
            me = [dst_sorted[estarts[s]:estarts[s + 1]] for s in moved]
            ml = np.array([len(x) for x in me])
            if ml.sum():
                mf = np.concatenate(me)
                ms = np.repeat(np.arange(len(moved)), ml)
                np.subtract.at(C, (mf, oldq[ms]), 1)
                np.add.at(C, (mf, newq[ms]), 1)
            applied += len(sa)
        if applied == 0:
            break
    return core_of, C


def preprocess(edge_index: np.ndarray, edge_attr: np.ndarray, batch: np.ndarray,
               c_scal) -> Prep:
    pr = Prep()
    src = edge_index[0].astype(np.int64)
    dst = edge_index[1].astype(np.int64)
    attr = edge_attr[:, 0].astype(np.float32)

    deg_in = np.bincount(dst, minlength=N).astype(np.int64)
    core_of, Cq = _balance_quarters(src, dst, deg_in)

    n_per_core = N // NCORES                      # 12500
    n_slots = (n_per_core + P - 1) // P           # 98
    n_loc = n_slots * P                           # 12544
    n_tab = NCORES * n_loc                        # 100352
    qrows = n_tab // NQ                           # 25088
    assert qrows <= 32767

    # within-core order: max per-quarter in-degree desc, then argmax quarter,
    # then profile — groups nodes whose worst quarter matches into the same
    # slot, which is what drives the per-(slot, quarter) column budget.
    mxq = Cq.max(axis=1)
    amq = np.argmax(Cq, axis=1)
    loc_of = np.empty(N, np.int64)
    for c in range(NCORES):
        ids = np.nonzero(core_of == c)[0]
        key = np.lexsort((ids, -Cq[ids, 3], -Cq[ids, 2], -Cq[ids, 1],
                          -Cq[ids, 0], amq[ids], -mxq[ids]))
        srt = ids[key]
        loc_of[srt] = np.arange(len(srt))
    slot_of = loc_of // P
    p_of = loc_of % P
    new_glob = core_of.astype(np.int64) * n_loc + loc_of

    # per-edge placement
    c_e = core_of[dst]
    s_e = slot_of[dst]
    p_e = p_of[dst]
    q_e = (core_of[src] // 2).astype(np.int64)
    # j = rank within (dst, q)
    ordk = np.lexsort((np.arange(E), q_e, dst))
    ds = dst[ordk]
    qs = q_e[ordk]
    grp = np.empty(E, bool)
    grp[0] = True
    grp[1:] = (ds[1:] != ds[:-1]) | (qs[1:] != qs[:-1])
    gid_idx = np.cumsum(grp) - 1
    first_pos = np.full(gid_idx[-1] + 1, E, np.int64)
    np.minimum.at(first_pos, gid_idx, np.arange(E))
    j_sorted = np.arange(E) - first_pos[gid_idx]
    j_e = np.empty(E, np.int64)
    j_e[ordk] = j_sorted

    # tiles per (slot, quarter): global max over cores & partitions
    tq = np.zeros((n_slots, NQ), np.int64)
    np.maximum.at(tq, (s_e, q_e), j_e + 1)
    for s in range(n_slots):
        if tq[s].sum() == 0:
            tq[s, 0] = 1
    cols_slot = tq.sum(axis=1)                     # [n_slots]
    col_off = np.zeros(n_slots + 1, np.int64)
    col_off[1:] = np.cumsum(cols_slot)
    TTC = int(col_off[-1])                         # total edge-columns

    # gather groups
    groups = []
    cur, cur_cols = [], 0
    for s in range(n_slots):
        if cur and (cur_cols + cols_slot[s] > GB_COLS or len(cur) >= GB_SLOTS):
            groups.append(cur)
            cur, cur_cols = [], 0
        cur.append(s)
        cur_cols += int(cols_slot[s])
    if cur:
        groups.append(cur)

    # per-(group, quarter) stream layout
    n_gb = len(groups)
    cols_gq = np.zeros((n_gb, NQ), np.int64)
    for g, sl in enumerate(groups):
        for q in range(NQ):
            cols_gq[g, q] = sum(int(tq[s, q]) for s in sl)
    # chs[(g, q, s)] = column offset of slot s's q-run inside group g's hs
    chs = {}
    base_gq = {}
    for g, sl in enumerate(groups):
        o = 0
        for q in range(NQ):
            base_gq[(g, q)] = o
            for s in sl:
                chs[(g, q, s)] = o
                o += int(tq[s, q])
    # sidx blob: per (g, q) contiguous ranges of idx columns ([128, cols*8])
    sidx_off = {}
    so = 0
    for g in range(n_gb):
        for q in range(NQ):
            sidx_off[(g, q)] = so
            so += int(cols_gq[g, q]) * 8
    SC = so

    # qoff[s][q] = column offset of q-run inside slot s's local cols
    qoff = np.zeros((n_slots, NQ + 1), np.int64)
    for s in range(n_slots):
        qoff[s, 1:] = np.cumsum(tq[s])

    # ---- build per-core streams & meta
    gb_of_slot = np.empty(n_slots, np.int64)
    for g, sl in enumerate(groups):
        for s in sl:
            gb_of_slot[s] = g
    # stream position of each edge inside its core's sidx flat array:
    # pos = sidx_off[(g, q)]*16 ... in IDX units: idxpos = (base of (g,q) in
    # idx units) + (chs[(g,q,s)] - base_gq[(g,q)] + j)*128 + p
    idx_base_gq = {k: v * 16 for k, v in sidx_off.items()}

    e_g = gb_of_slot[s_e]
    e_base = np.empty(E, np.int64)
    e_chs = np.empty(E, np.int64)
    e_bgq = np.empty(E, np.int64)
    # vectorize via lookup tables
    base_tab = np.zeros((n_gb, NQ), np.int64)
    for (g, q), v in idx_base_gq.items():
        base_tab[g, q] = v
    chs_tab = np.zeros((n_slots, NQ), np.int64)
    bgq_tab = np.zeros((n_gb, NQ), np.int64)
    for (g, q), v in base_gq.items():
        bgq_tab[g, q] = v
    for (g, q, s), v in chs.items():
        chs_tab[s, q] = v
    e_base = base_tab[e_g, q_e]
    e_chs = chs_tab[s_e, q_e]
    e_bgq = bgq_tab[e_g, q_e]
    e_idxpos = e_base + (e_chs - e_bgq + j_e) * P + p_e

    sidx = np.zeros((NCORES, P, SC), np.int16)
    for c in range(NCORES):
        m = c_e == c
        flat = np.zeros(SC * 16, np.int16)
        flat[e_idxpos[m]] = (new_glob[src[m]] - q_e[m] * qrows).astype(np.int16)
        sidx[c] = _wrap16(flat)

    # attr_eff meta per layer: [128, TTC] bf16, position (p, col_off[s]+qoff+j)
    e_col = col_off[s_e] + qoff[s_e, q_e] + j_e
    attr_m = np.zeros((3, NCORES, P, TTC), np.float32)
    attr_m[:] = PAD_NEG
    for c in range(NCORES):
        m = c_e == c
        for li in range(3):
            attr_m[li, c, p_e[m], e_col[m]] = attr[m] * np.float32(c_scal[li])
    # pad positions stay PAD_NEG; but positions beyond a slot's real edges in
    # partitions with fewer edges are also PAD_NEG (init).

    # graph pooling tables
    cnt_g = np.bincount(batch.astype(np.int64), minlength=G).astype(np.float32)
    wg = 1.0 / np.maximum(cnt_g, 1.0)
    g_of = batch.astype(np.int64)
    gidm = np.full((NCORES, n_loc), -1.0, np.float32)
    winv = np.zeros((NCORES, n_loc), np.float32)
    ids = np.arange(N)
    gidm[core_of[ids], loc_of[ids]] = g_of.astype(np.float32)
    winv[core_of[ids], loc_of[ids]] = wg[g_of]
    gidm = gidm.reshape(NCORES, n_slots, P).transpose(0, 2, 1)
    winv = winv.reshape(NCORES, n_slots, P).transpose(0, 2, 1)

    pr.n_slots, pr.n_loc, pr.n_tab, pr.qrows, pr.TTC = \
        n_slots, n_loc, n_tab, qrows, TTC
    pr.tq, pr.cols_slot, pr.col_off, pr.qoff = tq, cols_slot, col_off, qoff
    pr.groups, pr.cols_gq, pr.chs, pr.base_gq, pr.sidx_off, pr.SC = \
        groups, cols_gq, chs, base_gq, sidx_off, SC
    pr.sidx, pr.attr_m = sidx, attr_m
    pr.gid, pr.winv = gidm, winv
    pr.core_of, pr.loc_of, pr.new_glob = core_of, loc_of, new_glob
    return pr


def pack_weights(inputs: dict) -> dict:
    w = {}
    for l in (1, 2, 3):
        W = np.asarray(inputs[f"W{l}"], np.float32)
        a_s = np.asarray(inputs[f"as{l}"], np.float32)
        a_d = np.asarray(inputs[f"ad{l}"], np.float32)
        # x @ wext = [h | 0 | h@a_s | h@a_d]; the 0 col is overwritten with 1
        w[f"wext{l}"] = np.concatenate(
            [W, np.zeros((HID, 1), np.float32),
             (W @ a_s)[:, None], (W @ a_d)[:, None]], axis=1)
        w[f"c{l}"] = float(np.asarray(inputs[f"We{l}"], np.float32)[0]
                           @ np.asarray(inputs[f"ae{l}"], np.float32))
        w[f"brep{l}"] = np.tile(np.asarray(inputs[f"b{l}"], np.float32)[None, :],
                                (P, 1))
    w["wlin"] = np.asarray(inputs["Wlin"], np.float32)
    w["blin"] = float(np.asarray(inputs["blin"], np.float32)[0])
    return w


def host_table1(pr: Prep, w: dict, x: np.ndarray):
    """Layer-1 table [n_tab, RW] bf16 (row order new_glob) + adcol1 [c][P,S]."""
    t = x.astype(np.float32) @ w["wext1"]          # [N, 67]
    t[:, C_ONE] = 1.0
    T1 = np.zeros((pr.n_tab, RW), np.float32)
    T1[pr.new_glob[np.arange(N)], :TCOLS] = t
    adcol = np.zeros((NCORES, pr.n_loc), np.float32)
    adcol[pr.core_of, pr.loc_of] = t[:, C_AD]
    adcol = adcol.reshape(NCORES, pr.n_slots, P).transpose(0, 2, 1)
    return T1.astype(BF), adcol


# ----------------------------------------------------------------------------
# Device program
# ----------------------------------------------------------------------------

def build_program(pr: Prep):
    n_slots, n_loc, n_tab, qrows = pr.n_slots, pr.n_loc, pr.n_tab, pr.qrows
    tq, col_off, qoff = pr.tq, pr.col_off, pr.qoff
    groups, cols_gq, chs, base_gq, sidx_off = \
        pr.groups, pr.cols_gq, pr.chs, pr.base_gq, pr.sidx_off

    nc = bacc.Bacc("TRN2", target_bir_lowering=False, debug=False,
                   num_devices=NCORES)
    rg = [list(range(NCORES))]

    T1_d = nc.dram_tensor("T1", [n_tab, RW], BF16, kind="ExternalInput")
    sidx_d = nc.dram_tensor("sidx", [P, pr.SC], I16, kind="ExternalInput")
    attr_d = [nc.dram_tensor(f"attr{l}", [P, pr.TTC], BF16, kind="ExternalInput")
              for l in (1, 2, 3)]
    adcol1_d = nc.dram_tensor("adcol1", [P, n_slots], F32, kind="ExternalInput")
    gid_d = nc.dram_tensor("gid", [P, n_slots], F32, kind="ExternalInput")
    winv_d = nc.dram_tensor("winv", [P, n_slots], F32, kind="ExternalInput")
    wext_d = [nc.dram_tensor(f"wext{l}", [HID, TCOLS], F32, kind="ExternalInput")
              for l in (2, 3)]
    brep_d = [nc.dram_tensor(f"brep{l}", [P, HID], F32, kind="ExternalInput")
              for l in (1, 2, 3)]
    wlin_d = nc.dram_tensor("wlin", [HID, 1], F32, kind="ExternalInput")
    identb_d = nc.dram_tensor("identb", [P, P], BF16, kind="ExternalInput")
    iotg_d = nc.dram_tensor("iotg", [P, G], F32, kind="ExternalInput")
    ident_d = nc.dram_tensor("ident", [P, P], F32, kind="ExternalInput")
    out_d = nc.dram_tensor("out", [P, G // P], F32, kind="ExternalOutput")

    T_full = [None,
              nc.dram_tensor("T2", [n_tab, RW], BF16, kind="Internal",
                             addr_space="Shared"),
              nc.dram_tensor("T3", [n_tab, RW], BF16, kind="Internal",
                             addr_space="Shared")]
    T_sh = [None,
            nc.dram_tensor("Tsh2", [n_loc, RW], BF16, kind="Internal"),
            nc.dram_tensor("Tsh3", [n_loc, RW], BF16, kind="Internal")]

    with tile.TileContext(nc) as tc:
        with (
            tc.tile_pool(name="const", bufs=1) as cpool,
            tc.tile_pool(name="sbuf", bufs=4) as spool,
            tc.tile_pool(name="rs", bufs=8) as rpool,
            tc.tile_pool(name="gath", bufs=3) as gpool,
            tc.tile_pool(name="psum", bufs=2, space="PSUM") as ppool,
            tc.tile_pool(name="psum1", bufs=1, space="PSUM") as ppoolA,
            tc.tile_pool(name="ppool2", bufs=1, space="PSUM") as ppool1,
        ):
            identb_sb = cpool.tile([P, P], BF16, tag="identb")
            nc.sync.dma_start(out=identb_sb[:], in_=identb_d[:, :])
            ident_sb = cpool.tile([P, P], F32, tag="ident")
            nc.sync.dma_start(out=ident_sb[:], in_=ident_d[:, :])
            wext_sb = []
            for i in range(2):
                t1 = cpool.tile([HID, TCOLS], F32, tag=f"wext{i}", name=f"wext{i}")
                nc.sync.dma_start(out=t1[:], in_=wext_d[i][:, :])
                wext_sb.append(t1)
            brep_sb = []
            for i in range(3):
                t2 = cpool.tile([P, HID], F32, tag=f"brep{i}", name=f"brep{i}")
                nc.sync.dma_start(out=t2[:], in_=brep_d[i][:, :])
                brep_sb.append(t2)
            wlin_sb = cpool.tile([HID, 1], F32, tag="wlin")
            nc.sync.dma_start(out=wlin_sb[:], in_=wlin_d[:, :])
            gid_sb = cpool.tile([P, n_slots], F32, tag="gid")
            nc.sync.dma_start(out=gid_sb[:], in_=gid_d[:, :])
            winv_sb = cpool.tile([P, n_slots], F32, tag="winv")
            nc.sync.dma_start(out=winv_sb[:], in_=winv_d[:, :])
            iotg_sb = cpool.tile([P, G], F32, tag="iotg")
            nc.sync.dma_start(out=iotg_sb[:], in_=iotg_d[:, :])
            adcol_sb = [cpool.tile([P, n_slots], F32, tag=f"adcol{l}",
                                   name=f"adcol{l}") for l in range(3)]
            nc.sync.dma_start(out=adcol_sb[0][:], in_=adcol1_d[:, :])

            pool_ps = [ppool1.tile([P, HID], F32, tag=f"pool{h}", name=f"pool{h}")
                       for h in range(G // P)]

            for l in range(3):
                last = l == 2
                tab = T1_d if l == 0 else T_full[l]
                for g, sl in enumerate(groups):
                    gcols = int(cols_gq[g].sum())
                    hs = gpool.tile([P, gcols * RW], BF16, tag="hs",
                                    name=f"hs_{l}_{g}")
                    hs3 = hs[:].rearrange("p (t c) -> p t c", c=RW)
                    for q in range(NQ):
                        ncq = int(cols_gq[g, q])
                        if ncq == 0:
                            continue
                        o = sidx_off[(g, q)]
                        idx_sb = spool.tile([P, ncq * 8], I16, tag="sidx",
                                            name=f"sidx_{l}_{g}_{q}")
                        nc.sync.dma_start(out=idx_sb[:],
                                          in_=sidx_d[:, o:o + ncq * 8])
                        nidx = ncq * P
                        c0 = base_gq[(g, q)]
                        npieces = (nidx + PIECE - 1) // PIECE
                        per = ((nidx // P + npieces - 1) // npieces)  # cols
                        for pi in range(npieces):
                            ca = pi * per
                            cb = min(ncq, (pi + 1) * per)
                            if cb <= ca:
                                continue
                            nc.gpsimd.dma_gather(
                                out_ap=hs3[:, c0 + ca:c0 + cb, :],
                                in_ap=tab[q * qrows:(q + 1) * qrows, :],
                                idxs_ap=idx_sb[:, ca * 8:cb * 8],
                                num_idxs=(cb - ca) * P,
                                num_idxs_reg=(cb - ca) * P, elem_size=RW)

                    for s in sl:
                        t = int(pr.cols_slot[s])
                        o = int(col_off[s])
                        attr_sb = spool.tile([P, t], BF16, tag="attrm",
                                             name=f"attr_{l}_{s}")
                        nc.sync.dma_start(out=attr_sb[:],
                                          in_=attr_d[l][:, o:o + t])
                        # X = attr_eff + adcol[p] (+ asrc per quarter run)
                        X = spool.tile([P, t], F32, tag="xsum",
                                       name=f"X_{l}_{s}")
                        nc.vector.tensor_scalar(
                            out=X[:], in0=attr_sb[:],
                            scalar1=adcol_sb[l][:, s:s + 1],
                            scalar2=None,
                            op0=mybir.AluOpType.add)
                        for q in range(NQ):
                            nt = int(tq[s, q])
                            if nt == 0:
                                continue
                            cj = chs[(g, q, s)]
                            qo = int(qoff[s, q])
                            asrc_v = hs3[:, cj:cj + nt, C_AS:C_AS + 1] \
                                .rearrange("p t c -> p (t c)")
                            nc.vector.tensor_tensor(
                                out=X[:, qo:qo + nt], in0=X[:, qo:qo + nt],
                                in1=asrc_v, op=mybir.AluOpType.add)
                        alf = spool.tile([P, t], F32, tag="alf",
                                         name=f"alf_{l}_{s}")
                        nc.vector.scalar_tensor_tensor(
                            out=alf[:], in0=X[:], scalar=NEG_SLOPE,
                            in1=X[:], op0=mybir.AluOpType.mult,
                            op1=mybir.AluOpType.max)
                        ex = spool.tile([P, t], F32, tag="ex",
                                        name=f"ex_{l}_{s}")
                        nc.scalar.activation(out=ex[:], in_=alf[:],
                                             func=mybir.ActivationFunctionType.Exp)

                        agg = ppool.tile([P, C_ONE + 1], F32, tag="agg",
                                         name=f"agg_{l}_{s}")
                        nm = 0
                        for q in range(NQ):
                            nt = int(tq[s, q])
                            if nt == 0:
                                continue
                            cj = chs[(g, q, s)]
                            qo = int(qoff[s, q])
                            for k in range(nt):
                                rsc = rpool.tile([P, C_ONE + 1], BF16,
                                                 tag="rsc",
                                                 name=f"rsc_{l}_{s}_{qo + k}")
                                nc.vector.tensor_scalar(
                                    out=rsc[:],
                                    in0=hs3[:, cj + k, 0:C_ONE + 1],
                                    scalar1=ex[:, qo + k:qo + k + 1],
                                    scalar2=None,
                                    op0=mybir.AluOpType.mult)
                                nc.tensor.matmul(
                                    out=agg[:], lhsT=identb_sb[:], rhs=rsc[:],
                                    start=(nm == 0), stop=(nm == t - 1),
                                    skip_group_check=True)
                                nm += 1

                        # epilogue
                        dpe = spool.tile([P, 1], F32, tag="dpe",
                                         name=f"dpe_{l}_{s}")
                        nc.vector.tensor_scalar_add(
                            out=dpe[:], in0=agg[:, C_ONE:C_ONE + 1],
                            scalar1=EPS)
                        rcp = spool.tile([P, 1], F32, tag="rcp",
                                         name=f"rcp_{l}_{s}")
                        nc.vector.reciprocal(out=rcp[:], in_=dpe[:])
                        x2 = spool.tile([P, HID], F32, tag="x2",
                                        name=f"x2_{l}_{s}")
                        nc.scalar.activation(
                            out=x2[:], in_=agg[:, 0:C_ONE],
                            func=mybir.ActivationFunctionType.Copy,
                            scale=rcp[:, 0:1])
                        x2b = spool.tile([P, HID], F32, tag="x2b",
                                         name=f"x2b_{l}_{s}")
                        nc.vector.tensor_tensor(out=x2b[:], in0=x2[:],
                                                in1=brep_sb[l][:],
                                                op=mybir.AluOpType.add)
                        if not last:
                            x3 = spool.tile([P, HID], F32, tag="x3",
                                            name=f"x3_{l}_{s}")
                            nc.scalar.activation(
                                out=x3[:], in_=x2b[:],
                                func=mybir.ActivationFunctionType.Relu)
                            xt_ps = ppoolA.tile([HID, P], F32, tag="xtps")
                            nc.tensor.transpose(out=xt_ps[:], in_=x3[:],
                                                identity=ident_sb[:])
                            xt_sb = spool.tile([HID, P], F32, tag="xtsb",
                                               name=f"xt_{l}_{s}")
                            nc.scalar.copy(out=xt_sb[:], in_=xt_ps[:])
                            tn_ps = ppoolA.tile([P, TCOLS], F32, tag="tps")
                            nc.tensor.matmul(out=tn_ps[:], lhsT=xt_sb[:],
                                             rhs=wext_sb[l][:],
                                             start=True, stop=True)
                            nc.vector.tensor_copy(
                                out=adcol_sb[l + 1][:, s:s + 1],
                                in_=tn_ps[:, C_AD:C_AD + 1])
                            trow = spool.tile([P, RW], BF16, tag="trow",
                                              name=f"trow_{l}_{s}")
                            nc.scalar.copy(out=trow[:, 0:TCOLS], in_=tn_ps[:])
                            nc.vector.memset(trow[:, C_ONE:C_ONE + 1], 1.0)
                            nc.sync.dma_start(
                                out=T_sh[l + 1][s * P:(s + 1) * P, :],
                                in_=trow[:])
                        else:
                            for h in range(G // P):
                                gih = spool.tile([P, P], F32, tag="gih",
                                                 name=f"gi_{s}_{h}")
                                nc.vector.tensor_scalar(
                                    out=gih[:],
                                    in0=iotg_sb[:, h * P:(h + 1) * P],
                                    scalar1=gid_sb[:, s:s + 1],
                                    scalar2=winv_sb[:, s:s + 1],
                                    op0=mybir.AluOpType.is_equal,
                                    op1=mybir.AluOpType.mult)
                                nc.tensor.matmul(
                                    out=pool_ps[h][:], lhsT=gih[:], rhs=x2b[:],
                                    start=(s == 0), stop=(s == n_slots - 1),
                                    skip_group_check=True)

                if not last:
                    nc.gpsimd.collective_compute(
                        "AllGather", mybir.AluOpType.bypass, replica_groups=rg,
                        ins=[T_sh[l + 1].ap().opt()],
                        outs=[T_full[l + 1].ap().opt()])

            # ---- head
            out_sb = spool.tile([P, G // P], F32, tag="outsb")
            for h in range(G // P):
                pool_sb = spool.tile([P, HID], F32, tag="poolsb",
                                     name=f"poolsb{h}")
                nc.vector.tensor_copy(out=pool_sb[:], in_=pool_ps[h][:])
                pt_ps = ppoolA.tile([HID, P], F32, tag="xtps")
                nc.tensor.transpose(out=pt_ps[:], in_=pool_sb[:],
                                    identity=ident_sb[:])
                pt_sb = spool.tile([HID, P], F32, tag="xtsb", name=f"ptsb{h}")
                nc.scalar.copy(out=pt_sb[:], in_=pt_ps[:])
                o_ps = ppoolA.tile([P, 1], F32, tag="tps", name=f"o_ps{h}")
                nc.tensor.matmul(out=o_ps[:], lhsT=pt_sb[:], rhs=wlin_sb[:],
                                 start=True, stop=True)
                nc.vector.tensor_copy(out=out_sb[:, h:h + 1], in_=o_ps[:])
            nc.sync.dma_start(out=out_d[:, :], in_=out_sb[:])

    nc.compile()
    return nc


# ----------------------------------------------------------------------------
# Entry point
# ----------------------------------------------------------------------------

def make_inmaps(pr: Prep, w: dict, T1, adcol1):
    identb = np.eye(P, dtype=np.float32).astype(BF)
    ident = np.eye(P, dtype=np.float32)
    iotg = np.tile(np.arange(G, dtype=np.float32)[None, :], (P, 1))
    in_maps = []
    for c in range(NCORES):
        m = {
            "T1": T1,
            "sidx": pr.sidx[c],
            "adcol1": adcol1[c],
            "gid": pr.gid[c],
            "winv": pr.winv[c],
            "wlin": w["wlin"],
            "identb": identb,
            "ident": ident,
            "iotg": iotg,
        }
        for li, l in enumerate((1, 2, 3)):
            m[f"attr{l}"] = pr.attr_m[li, c].astype(BF)
            m[f"brep{l}"] = w[f"brep{l}"]
        for l in (2, 3):
            m[f"wext{l}"] = w[f"wext{l}"]
        in_maps.append(m)
    return in_maps


def kernel(**inputs) -> np.ndarray:
    inputs = {k: np.asarray(v) for k, v in inputs.items()}
    w = pack_weights(inputs)
    pr = preprocess(inputs["edge_index"], inputs["edge_attr"], inputs["batch"],
                    [w["c1"], w["c2"], w["c3"]])
    T1, adcol1 = host_table1(pr, w, np.asarray(inputs["x"], np.float32))
    nc = build_program(pr)
    in_maps = make_inmaps(pr, w, T1, adcol1)
    res = bass_utils.run_bass_kernel_spmd(nc, in_maps,
                                          core_ids=list(range(NCORES)))
    out = np.zeros(G, np.float64)
    for c in range(NCORES):
        oc = res.results[c]["out"]
        out += oc.T.reshape(-1).astype(np.float64)
    return (out + w["blin"]).astype(np.float32)


# revision 11
# speedup vs baseline: 1.7839x; 1.1847x over previous
"""3-layer GAT + mean-pool + linear head on 8 Trainium2 NeuronCores.

V2 strategy (dst-partition-aligned edge layout, no dst gather):
  - Host: nodes dealt to 8 cores by in-degree round-robin, then a vectorized
    local-search pass swaps equal-degree nodes between cores to even out each
    destination's in-edge spread across the 4 source table quarters (quarter =
    core pair, forced by the AllGather row order and int16 gather indices).
  - Within a core, nodes sort by in-degree into 128-node slots; an edge lives
    at partition p = its dst's slot position, column j = its rank among that
    (dst, quarter)'s edges.  a_dst therefore becomes a per-partition scalar
    (adcol), eliminating the per-edge dst gather entirely.
  - Per layer a table T[node] = [h | 1 | h@a_s | h@a_d | pad] (bf16, 256B
    rows) is the gather source.  Layer 1's table is precomputed on the host
    and shipped as an input (no AllGather).  Layers 2/3 build their shard
    during the previous layer's epilogue and AllGather it.
  - Per slot: ex = exp(leakyrelu(asrc[src] + adcol[p] + c*attr)) (bf16); per
    edge-column the gathered row [h|1] is scaled by ex (DVE, 4x bf16 mode)
    and accumulated into PSUM [128, 65] by a matmul against a constant
    identity (lhsT), yielding [sum ex*h | sum ex] per dst node.
  - Epilogue: x' = x2 + b with x2 = num/(den+eps); next-layer table rows via
    PE transpose + matmul with packed weights; graph mean-pool via PE with a
    host-built (1/cnt)-weighted graph indicator; host sums per-core partials.
"""

import sys

for _p in ("/opt/trn_rl_repo",):
    if _p not in sys.path:
        sys.path.insert(0, _p)

import numpy as np
import ml_dtypes

import concourse.bacc as bacc
import concourse.bass as bass
import concourse.tile as tile
from concourse import bass_utils, mybir

# Problem constants (hardcoded per spec)
N = 100_000
E = 1_600_000
G = 256
HID = 64
NEG_SLOPE = 0.2
EPS = 1e-16

NCORES = 8
NQ = 4             # src table quarters (int16 gather index limit)
P = 128            # partitions / node-block size
RW = 128           # table row width (bf16 cols) -> 256 bytes
C_ONE = 64         # constant-1 column (denominator rides the matmul)
C_AS = 65          # a_src column
C_AD = 66          # a_dst column
TCOLS = 67         # meaningful table columns
PAD_NEG = -1.0e30  # attr_eff value on pad edges -> ex = exp(-inf) = 0

GB_COLS = 96       # target gathered columns per gather batch
GB_SLOTS = 8       # max slots per gather batch
PIECE = 1024       # max indices per dma_gather instruction (HW SWDGE ring limit)

F32 = mybir.dt.float32
BF16 = mybir.dt.bfloat16
I16 = mybir.dt.int16

BF = ml_dtypes.bfloat16

N_SWEEPS = 12      # quarter-balance local search sweeps


class Prep:
    pass


# ----------------------------------------------------------------------------
# Host-side graph preprocessing
# ----------------------------------------------------------------------------

def _wrap16(flat: np.ndarray) -> np.ndarray:
    """int16 stream -> [128, n/16] wrapped layout (k at [k%16, k//16], x8)."""
    n = flat.shape[0]
    assert n % 16 == 0
    w = flat.reshape(n // 16, 16).T          # [16, n/16]
    return np.tile(w, (8, 1))                # [128, n/16]


def _balance_quarters(src, dst, deg_in):
    """Assign nodes to cores (equal counts, aligned degree profiles) while
    evening out each dst's in-edge spread across the 4 quarters.  Batched
    local search with live counts + re-verification.  Returns (core_of, C)
    where C[n, q] = in-edges of n from quarter q."""
    ceil4 = (deg_in + 3) // 4
    nodes = np.arange(N)
    order = np.argsort(-deg_in, kind="stable")
    core_of = np.empty(N, np.int32)
    core_of[order] = np.arange(N) % NCORES

    eorder = np.argsort(src, kind="stable")
    estarts = np.searchsorted(src[eorder], np.arange(N + 1))
    dst_sorted = dst[eorder]

    C = np.zeros((N, NQ), np.int32)
    np.add.at(C, (dst, core_of[src] // 2), 1)
    for _sw in range(N_SWEEPS):
        q_of = (core_of // 2).astype(np.int64)
        addpen = (C >= ceil4[:, None])
        decgain = (C > ceil4[:, None])
        A = np.zeros((N, NQ), np.int64)
        for q in range(NQ):
            np.add.at(A[:, q], src, addpen[dst, q])
        B = np.zeros(N, np.int64)
        np.add.at(B, src, decgain[dst, q_of[src]])
        A2 = A.copy()
        A2[nodes, q_of] = 10**9
        best = np.argmin(A2, axis=1).astype(np.int64)
        gain = B - A2[nodes, best]
        movers = nodes[gain > 0]
        if len(movers) == 0:
            break
        movers = movers[np.argsort(-gain[movers], kind="stable")]
        applied = 0
        for a0 in range(0, len(movers), 2048):
            ch = movers[a0:a0 + 2048]
            eds = [dst_sorted[estarts[s]:estarts[s + 1]] for s in ch]
            lens = np.array([len(e) for e in eds])
            if lens.sum() == 0:
                continue
            flat = np.concatenate(eds)
            seg = np.repeat(np.arange(len(ch)), lens)
            qcur = (core_of[ch] // 2).astype(np.int64)
            Bv = np.zeros(len(ch), np.int64)
            np.add.at(Bv, seg, (C[flat, qcur[seg]] > ceil4[flat]))
            Av = np.zeros((len(ch), NQ), np.int64)
            for q in range(NQ):
                np.add.at(Av[:, q], seg, (C[flat, q] >= ceil4[flat]))
            Av[np.arange(len(ch)), qcur] = 10**9
            bq = np.argmin(Av, axis=1)
            gv = Bv - Av[np.arange(len(ch)), bq]
            ok = gv > 0
            if not ok.any():
                continue
            ids = ch[ok]
            icur = qcur[ok]
            ibest = bq[ok]
            ideg = deg_in[ids]
            lo = np.minimum(icur, ibest)
            hi = np.maximum(icur, ibest)
            fwd = icur == lo
            key = (ideg * 16 + lo * 4 + hi) * 2 + fwd
            srt = np.argsort(key, kind="stable")
            ids, key, fwd = ids[srt], key[srt], fwd[srt]
            uk, st2 = np.unique(key >> 1, return_index=True)
            st2 = list(st2) + [len(key)]
            swap_a, swap_b = [], []
            for gi in range(len(uk)):
                b0, b1 = st2[gi], st2[gi + 1]
                ids_f = ids[b0:b1][fwd[b0:b1]]
                ids_b = ids[b0:b1][~fwd[b0:b1]]
                m = min(len(ids_f), len(ids_b))
                if m:
                    swap_a.append(ids_f[:m])
                    swap_b.append(ids_b[:m])
            if not swap_a:
                continue
            sa = np.concatenate(swap_a)
            sb = np.concatenate(swap_b)
            oq_a = (core_of[sa] // 2).astype(np.int64)
            oq_b = (core_of[sb] // 2).astype(np.int64)
            tmp = core_of[sa].copy()
            core_of[sa] = core_of[sb]
            core_of[sb] = tmp
            moved = np.concatenate([sa, sb])
            oldq = np.concatenate([oq_a, oq_b])
            newq = np.concatenate([oq_b, oq_a])
            me = [dst_sorted[estarts[s]:estarts[s + 1]] for s in moved]
            ml = np.array([len(x) for x in me])
            if ml.sum():
                mf = np.concatenate(me)
                ms = np.repeat(np.arange(len(moved)), ml)
                np.subtract.at(C, (mf, oldq[ms]), 1)
                np.add.at(C, (mf, newq[ms]), 1)
            applied += len(sa)
        if applied == 0:
            break
    return core_of, C


def preprocess(edge_index: np.ndarray, edge_attr: np.ndarray, batch: np.ndarray,
               c_scal) -> Prep:
    pr = Prep()
    src = edge_index[0].astype(np.int64)
    dst = edge_index[1].astype(np.int64)
    attr = edge_attr[:, 0].astype(np.float32)

    deg_in = np.bincount(dst, minlength=N).astype(np.int64)
    core_of, Cq = _balance_quarters(src, dst, deg_in)

    n_per_core = N // NCORES                      # 12500
    n_slots = (n_per_core + P - 1) // P           # 98
    n_loc = n_slots * P                           # 12544
    n_tab = NCORES * n_loc                        # 100352
    qrows = n_tab // NQ                           # 25088
    assert qrows <= 32767

    # within-core order: max per-quarter in-degree desc, then argmax quarter,
    # then profile — groups nodes whose worst quarter matches into the same
    # slot, which is what drives the per-(slot, quarter) column budget.
    mxq = Cq.max(axis=1)
    amq = np.argmax(Cq, axis=1)
    loc_of = np.empty(N, np.int64)
    for c in range(NCORES):
        ids = np.nonzero(core_of == c)[0]
        key = np.lexsort((ids, -Cq[ids, 3], -Cq[ids, 2], -Cq[ids, 1],
                          -Cq[ids, 0], amq[ids], -mxq[ids]))
        srt = ids[key]
        loc_of[srt] = np.arange(len(srt))
    slot_of = loc_of // P
    p_of = loc_of % P
    new_glob = core_of.astype(np.int64) * n_loc + loc_of

    # per-edge placement
    c_e = core_of[dst]
    s_e = slot_of[dst]
    p_e = p_of[dst]
    q_e = (core_of[src] // 2).astype(np.int64)
    # j = rank within (dst, q)
    ordk = np.lexsort((np.arange(E), q_e, dst))
    ds = dst[ordk]
    qs = q_e[ordk]
    grp = np.empty(E, bool)
    grp[0] = True
    grp[1:] = (ds[1:] != ds[:-1]) | (qs[1:] != qs[:-1])
    gid_idx = np.cumsum(grp) - 1
    first_pos = np.full(gid_idx[-1] + 1, E, np.int64)
    np.minimum.at(first_pos, gid_idx, np.arange(E))
    j_sorted = np.arange(E) - first_pos[gid_idx]
    j_e = np.empty(E, np.int64)
    j_e[ordk] = j_sorted

    # tiles per (slot, quarter): global max over cores & partitions
    tq = np.zeros((n_slots, NQ), np.int64)
    np.maximum.at(tq, (s_e, q_e), j_e + 1)
    for s in range(n_slots):
        if tq[s].sum() == 0:
            tq[s, 0] = 1
    cols_slot = tq.sum(axis=1)                     # [n_slots]
    col_off = np.zeros(n_slots + 1, np.int64)
    col_off[1:] = np.cumsum(cols_slot)
    TTC = int(col_off[-1])                         # total edge-columns

    # gather groups
    groups = []
    cur, cur_cols = [], 0
    for s in range(n_slots):
        if cur and (cur_cols + cols_slot[s] > GB_COLS or len(cur) >= GB_SLOTS):
            groups.append(cur)
            cur, cur_cols = [], 0
        cur.append(s)
        cur_cols += int(cols_slot[s])
    if cur:
        groups.append(cur)

    # per-(group, quarter) stream layout
    n_gb = len(groups)
    cols_gq = np.zeros((n_gb, NQ), np.int64)
    for g, sl in enumerate(groups):
        for q in range(NQ):
            cols_gq[g, q] = sum(int(tq[s, q]) for s in sl)
    # chs[(g, q, s)] = column offset of slot s's q-run inside group g's hs
    chs = {}
    base_gq = {}
    for g, sl in enumerate(groups):
        o = 0
        for q in range(NQ):
            base_gq[(g, q)] = o
            for s in sl:
                chs[(g, q, s)] = o
                o += int(tq[s, q])
    # sidx blob: per (g, q) contiguous ranges of idx columns ([128, cols*8])
    sidx_off = {}
    so = 0
    for g in range(n_gb):
        for q in range(NQ):
            sidx_off[(g, q)] = so
            so += int(cols_gq[g, q]) * 8
    SC = so

    # qoff[s][q] = column offset of q-run inside slot s's local cols
    qoff = np.zeros((n_slots, NQ + 1), np.int64)
    for s in range(n_slots):
        qoff[s, 1:] = np.cumsum(tq[s])

    # ---- build per-core streams & meta
    gb_of_slot = np.empty(n_slots, np.int64)
    for g, sl in enumerate(groups):
        for s in sl:
            gb_of_slot[s] = g
    # stream position of each edge inside its core's sidx flat array:
    # pos = sidx_off[(g, q)]*16 ... in IDX units: idxpos = (base of (g,q) in
    # idx units) + (chs[(g,q,s)] - base_gq[(g,q)] + j)*128 + p
    idx_base_gq = {k: v * 16 for k, v in sidx_off.items()}

    e_g = gb_of_slot[s_e]
    e_base = np.empty(E, np.int64)
    e_chs = np.empty(E, np.int64)
    e_bgq = np.empty(E, np.int64)
    # vectorize via lookup tables
    base_tab = np.zeros((n_gb, NQ), np.int64)
    for (g, q), v in idx_base_gq.items():
        base_tab[g, q] = v
    chs_tab = np.zeros((n_slots, NQ), np.int64)
    bgq_tab = np.zeros((n_gb, NQ), np.int64)
    for (g, q), v in base_gq.items():
        bgq_tab[g, q] = v
    for (g, q, s), v in chs.items():
        chs_tab[s, q] = v
    e_base = base_tab[e_g, q_e]
    e_chs = chs_tab[s_e, q_e]
    e_bgq = bgq_tab[e_g, q_e]
    e_idxpos = e_base + (e_chs - e_bgq + j_e) * P + p_e

    sidx = np.zeros((NCORES, P, SC), np.int16)
    for c in range(NCORES):
        m = c_e == c
        flat = np.zeros(SC * 16, np.int16)
        flat[e_idxpos[m]] = (new_glob[src[m]] - q_e[m] * qrows).astype(np.int16)
        sidx[c] = _wrap16(flat)

    # attr_eff meta per layer: [128, TTC] bf16, position (p, col_off[s]+qoff+j)
    e_col = col_off[s_e] + qoff[s_e, q_e] + j_e
    attr_m = np.zeros((3, NCORES, P, TTC), np.float32)
    attr_m[:] = PAD_NEG
    for c in range(NCORES):
        m = c_e == c
        for li in range(3):
            attr_m[li, c, p_e[m], e_col[m]] = attr[m] * np.float32(c_scal[li])
    # pad positions stay PAD_NEG; but positions beyond a slot's real edges in
    # partitions with fewer edges are also PAD_NEG (init).

    # graph pooling tables
    cnt_g = np.bincount(batch.astype(np.int64), minlength=G).astype(np.float32)
    wg = 1.0 / np.maximum(cnt_g, 1.0)
    g_of = batch.astype(np.int64)
    gidm = np.full((NCORES, n_loc), -1.0, np.float32)
    winv = np.zeros((NCORES, n_loc), np.float32)
    ids = np.arange(N)
    gidm[core_of[ids], loc_of[ids]] = g_of.astype(np.float32)
    winv[core_of[ids], loc_of[ids]] = wg[g_of]
    gidm = gidm.reshape(NCORES, n_slots, P).transpose(0, 2, 1)
    winv = winv.reshape(NCORES, n_slots, P).transpose(0, 2, 1)

    pr.n_slots, pr.n_loc, pr.n_tab, pr.qrows, pr.TTC = \
        n_slots, n_loc, n_tab, qrows, TTC
    pr.tq, pr.cols_slot, pr.col_off, pr.qoff = tq, cols_slot, col_off, qoff
    pr.groups, pr.cols_gq, pr.chs, pr.base_gq, pr.sidx_off, pr.SC = \
        groups, cols_gq, chs, base_gq, sidx_off, SC
    pr.sidx, pr.attr_m = sidx, attr_m
    pr.gid, pr.winv = gidm, winv
    pr.core_of, pr.loc_of, pr.new_glob = core_of, loc_of, new_glob
    return pr


def pack_weights(inputs: dict) -> dict:
    w = {}
    for l in (1, 2, 3):
        W = np.asarray(inputs[f"W{l}"], np.float32)
        a_s = np.asarray(inputs[f"as{l}"], np.float32)
        a_d = np.asarray(inputs[f"ad{l}"], np.float32)
        # x @ wext = [h | 0 | h@a_s | h@a_d]; the 0 col is overwritten with 1
        w[f"wext{l}"] = np.concatenate(
            [W, np.zeros((HID, 1), np.float32),
             (W @ a_s)[:, None], (W @ a_d)[:, None]], axis=1)
        w[f"c{l}"] = float(np.asarray(inputs[f"We{l}"], np.float32)[0]
                           @ np.asarray(inputs[f"ae{l}"], np.float32))
        w[f"brep{l}"] = np.tile(np.asarray(inputs[f"b{l}"], np.float32)[None, :],
                                (P, 1))
    w["wlin"] = np.asarray(inputs["Wlin"], np.float32)
    w["blin"] = float(np.asarray(inputs["blin"], np.float32)[0])
    return w


def host_table1(pr: Prep, w: dict, x: np.ndarray):
    """Layer-1 table [n_tab, RW] bf16 (row order new_glob) + adcol1 [c][P,S]."""
    t = x.astype(np.float32) @ w["wext1"]          # [N, 67]
    t[:, C_ONE] = 1.0
    T1 = np.zeros((pr.n_tab, RW), np.float32)
    T1[pr.new_glob[np.arange(N)], :TCOLS] = t
    adcol = np.zeros((NCORES, pr.n_loc), np.float32)
    adcol[pr.core_of, pr.loc_of] = t[:, C_AD]
    adcol = adcol.reshape(NCORES, pr.n_slots, P).transpose(0, 2, 1)
    return T1.astype(BF), adcol


# ----------------------------------------------------------------------------
# Device program
# ----------------------------------------------------------------------------

def build_program(pr: Prep):
    n_slots, n_loc, n_tab, qrows = pr.n_slots, pr.n_loc, pr.n_tab, pr.qrows
    tq, col_off, qoff = pr.tq, pr.col_off, pr.qoff
    groups, cols_gq, chs, base_gq, sidx_off = \
        pr.groups, pr.cols_gq, pr.chs, pr.base_gq, pr.sidx_off

    nc = bacc.Bacc("TRN2", target_bir_lowering=False, debug=False,
                   num_devices=NCORES)
    rg = [list(range(NCORES))]

    T1_d = nc.dram_tensor("T1", [n_tab, RW], BF16, kind="ExternalInput")
    sidx_d = nc.dram_tensor("sidx", [P, pr.SC], I16, kind="ExternalInput")
    attr_d = [nc.dram_tensor(f"attr{l}", [P, pr.TTC], BF16, kind="ExternalInput")
              for l in (1, 2, 3)]
    adcol1_d = nc.dram_tensor("adcol1", [P, n_slots], F32, kind="ExternalInput")
    gid_d = nc.dram_tensor("gid", [P, n_slots], F32, kind="ExternalInput")
    winv_d = nc.dram_tensor("winv", [P, n_slots], F32, kind="ExternalInput")
    wext_d = [nc.dram_tensor(f"wext{l}", [HID, TCOLS], F32, kind="ExternalInput")
              for l in (2, 3)]
    brep_d = [nc.dram_tensor(f"brep{l}", [P, HID], F32, kind="ExternalInput")
              for l in (1, 2, 3)]
    wlin_d = nc.dram_tensor("wlin", [HID, 1], F32, kind="ExternalInput")
    identb_d = nc.dram_tensor("identb", [P, P], BF16, kind="ExternalInput")
    iotg_d = nc.dram_tensor("iotg", [P, G], F32, kind="ExternalInput")
    ident_d = nc.dram_tensor("ident", [P, P], F32, kind="ExternalInput")
    out_d = nc.dram_tensor("out", [P, G // P], F32, kind="ExternalOutput")

    T_full = [None,
              nc.dram_tensor("T2", [n_tab, RW], BF16, kind="Internal",
                             addr_space="Shared"),
              nc.dram_tensor("T3", [n_tab, RW], BF16, kind="Internal",
                             addr_space="Shared")]
    T_sh = [None,
            nc.dram_tensor("Tsh2", [n_loc, RW], BF16, kind="Internal"),
            nc.dram_tensor("Tsh3", [n_loc, RW], BF16, kind="Internal")]

    with tile.TileContext(nc) as tc:
        with (
            tc.tile_pool(name="const", bufs=1) as cpool,
            tc.tile_pool(name="sbuf", bufs=4) as spool,
            tc.tile_pool(name="rs", bufs=8) as rpool,
            tc.tile_pool(name="gath", bufs=3) as gpool,
            tc.tile_pool(name="psum", bufs=2, space="PSUM") as ppool,
            tc.tile_pool(name="psum1", bufs=1, space="PSUM") as ppoolA,
            tc.tile_pool(name="ppool2", bufs=1, space="PSUM") as ppool1,
        ):
            identb_sb = cpool.tile([P, P], BF16, tag="identb")
            nc.sync.dma_start(out=identb_sb[:], in_=identb_d[:, :])
            ident_sb = cpool.tile([P, P], F32, tag="ident")
            nc.sync.dma_start(out=ident_sb[:], in_=ident_d[:, :])
            wext_sb = []
            for i in range(2):
                t1 = cpool.tile([HID, TCOLS], F32, tag=f"wext{i}", name=f"wext{i}")
                nc.sync.dma_start(out=t1[:], in_=wext_d[i][:, :])
                wext_sb.append(t1)
            brep_sb = []
            for i in range(3):
                t2 = cpool.tile([P, HID], F32, tag=f"brep{i}", name=f"brep{i}")
                nc.sync.dma_start(out=t2[:], in_=brep_d[i][:, :])
                brep_sb.append(t2)
            wlin_sb = cpool.tile([HID, 1], F32, tag="wlin")
            nc.sync.dma_start(out=wlin_sb[:], in_=wlin_d[:, :])
            gid_sb = cpool.tile([P, n_slots], F32, tag="gid")
            nc.sync.dma_start(out=gid_sb[:], in_=gid_d[:, :])
            winv_sb = cpool.tile([P, n_slots], F32, tag="winv")
            nc.sync.dma_start(out=winv_sb[:], in_=winv_d[:, :])
            iotg_sb = cpool.tile([P, G], F32, tag="iotg")
            nc.sync.dma_start(out=iotg_sb[:], in_=iotg_d[:, :])
            adcol_sb = [cpool.tile([P, n_slots], F32, tag=f"adcol{l}",
                                   name=f"adcol{l}") for l in range(3)]
            nc.sync.dma_start(out=adcol_sb[0][:], in_=adcol1_d[:, :])

            pool_ps = [ppool1.tile([P, HID], F32, tag=f"pool{h}", name=f"pool{h}")
                       for h in range(G // P)]

            for l in range(3):
                last = l == 2
                tab = T1_d if l == 0 else T_full[l]
                for g, sl in enumerate(groups):
                    gcols = int(cols_gq[g].sum())
                    hs = gpool.tile([P, gcols * RW], BF16, tag="hs",
                                    name=f"hs_{l}_{g}")
                    hs3 = hs[:].rearrange("p (t c) -> p t c", c=RW)
                    for q in range(NQ):
                        ncq = int(cols_gq[g, q])
                        if ncq == 0:
                            continue
                        o = sidx_off[(g, q)]
                        idx_sb = spool.tile([P, ncq * 8], I16, tag="sidx",
                                            name=f"sidx_{l}_{g}_{q}")
                        nc.sync.dma_start(out=idx_sb[:],
                                          in_=sidx_d[:, o:o + ncq * 8])
                        nidx = ncq * P
                        c0 = base_gq[(g, q)]
                        npieces = (nidx + PIECE - 1) // PIECE
                        per = ((nidx // P + npieces - 1) // npieces)  # cols
                        for pi in range(npieces):
                            ca = pi * per
                            cb = min(ncq, (pi + 1) * per)
                            if cb <= ca:
                                continue
                            nc.gpsimd.dma_gather(
                                out_ap=hs3[:, c0 + ca:c0 + cb, :],
                                in_ap=tab[q * qrows:(q + 1) * qrows, :],
                                idxs_ap=idx_sb[:, ca * 8:cb * 8],
                                num_idxs=(cb - ca) * P,
                                num_idxs_reg=(cb - ca) * P, elem_size=RW)

                    for s in sl:
                        t = int(pr.cols_slot[s])
                        o = int(col_off[s])
                        attr_sb = spool.tile([P, t], BF16, tag="attrm",
                                             name=f"attr_{l}_{s}")
                        nc.sync.dma_start(out=attr_sb[:],
                                          in_=attr_d[l][:, o:o + t])
                        # X = attr_eff + adcol[p] (+ asrc per quarter run)
                        X = spool.tile([P, t], F32, tag="xsum",
                                       name=f"X_{l}_{s}")
                        nc.vector.tensor_scalar(
                            out=X[:], in0=attr_sb[:],
                            scalar1=adcol_sb[l][:, s:s + 1],
                            scalar2=None,
                            op0=mybir.AluOpType.add)
                        for q in range(NQ):
                            nt = int(tq[s, q])
                            if nt == 0:
                                continue
                            cj = chs[(g, q, s)]
                            qo = int(qoff[s, q])
                            asrc_v = hs3[:, cj:cj + nt, C_AS:C_AS + 1] \
                                .rearrange("p t c -> p (t c)")
                            nc.vector.tensor_tensor(
                                out=X[:, qo:qo + nt], in0=X[:, qo:qo + nt],
                                in1=asrc_v, op=mybir.AluOpType.add)
                        alf = spool.tile([P, t], F32, tag="alf",
                                         name=f"alf_{l}_{s}")
                        nc.vector.scalar_tensor_tensor(
                            out=alf[:], in0=X[:], scalar=NEG_SLOPE,
                            in1=X[:], op0=mybir.AluOpType.mult,
                            op1=mybir.AluOpType.max)
                        ex = spool.tile([P, t], F32, tag="ex",
                                        name=f"ex_{l}_{s}")
                        nc.scalar.activation(out=ex[:], in_=alf[:],
                                             func=mybir.ActivationFunctionType.Exp)

                        agg = ppool.tile([P, C_ONE + 1], F32, tag="agg",
                                         name=f"agg_{l}_{s}")
                        nm = 0
                        for q in range(NQ):
                            nt = int(tq[s, q])
                            if nt == 0:
                                continue
                            cj = chs[(g, q, s)]
                            qo = int(qoff[s, q])
                            for k in range(nt):
                                rsc = rpool.tile([P, C_ONE + 1], BF16,
                                                 tag="rsc",
                                                 name=f"rsc_{l}_{s}_{qo + k}")
                                nc.vector.tensor_scalar(
                                    out=rsc[:],
                                    in0=hs3[:, cj + k, 0:C_ONE + 1],
                                    scalar1=ex[:, qo + k:qo + k + 1],
                                    scalar2=None,
                                    op0=mybir.AluOpType.mult)
                                nc.tensor.matmul(
                                    out=agg[:], lhsT=identb_sb[:], rhs=rsc[:],
                                    start=(nm == 0), stop=(nm == t - 1),
                                    skip_group_check=True)
                                nm += 1

                        # epilogue
                        dpe = spool.tile([P, 1], F32, tag="dpe",
                                         name=f"dpe_{l}_{s}")
                        nc.vector.tensor_scalar_add(
                            out=dpe[:], in0=agg[:, C_ONE:C_ONE + 1],
                            scalar1=EPS)
                        rcp = spool.tile([P, 1], F32, tag="rcp",
                                         name=f"rcp_{l}_{s}")
                        nc.vector.reciprocal(out=rcp[:], in_=dpe[:])
                        x2 = spool.tile([P, HID], F32, tag="x2",
                                        name=f"x2_{l}_{s}")
                        nc.scalar.activation(
                            out=x2[:], in_=agg[:, 0:C_ONE],
                            func=mybir.ActivationFunctionType.Copy,
                            scale=rcp[:, 0:1])
                        x2b = spool.tile([P, HID], F32, tag="x2b",
                                         name=f"x2b_{l}_{s}")
                        nc.vector.tensor_tensor(out=x2b[:], in0=x2[:],
                                                in1=brep_sb[l][:],
                                                op=mybir.AluOpType.add)
                        if not last:
                            x3 = spool.tile([P, HID], F32, tag="x3",
                                            name=f"x3_{l}_{s}")
                            nc.scalar.activation(
                                out=x3[:], in_=x2b[:],
                                func=mybir.ActivationFunctionType.Relu)
                            xt_ps = ppoolA.tile([HID, P], F32, tag="xtps")
                            nc.tensor.transpose(out=xt_ps[:], in_=x3[:],
                                                identity=ident_sb[:])
                            xt_sb = spool.tile([HID, P], F32, tag="xtsb",
                                               name=f"xt_{l}_{s}")
                            nc.scalar.copy(out=xt_sb[:], in_=xt_ps[:])
                            tn_ps = ppoolA.tile([P, TCOLS], F32, tag="tps")
                            nc.tensor.matmul(out=tn_ps[:], lhsT=xt_sb[:],
                                             rhs=wext_sb[l][:],
                                             start=True, stop=True)
                            nc.vector.tensor_copy(
                                out=adcol_sb[l + 1][:, s:s + 1],
                                in_=tn_ps[:, C_AD:C_AD + 1])
                            trow = spool.tile([P, RW], BF16, tag="trow",
                                              name=f"trow_{l}_{s}")
                            nc.scalar.copy(out=trow[:, 0:TCOLS], in_=tn_ps[:])
                            nc.vector.memset(trow[:, C_ONE:C_ONE + 1], 1.0)
                            nc.sync.dma_start(
                                out=T_sh[l + 1][s * P:(s + 1) * P, :],
                                in_=trow[:])
                        else:
                            for h in range(G // P):
                                gih = spool.tile([P, P], F32, tag="gih",
                                                 name=f"gi_{s}_{h}")
                                nc.vector.tensor_scalar(
                                    out=gih[:],
                                    in0=iotg_sb[:, h * P:(h + 1) * P],
                                    scalar1=gid_sb[:, s:s + 1],
                                    scalar2=winv_sb[:, s:s + 1],
                                    op0=mybir.AluOpType.is_equal,
                                    op1=mybir.AluOpType.mult)
                                nc.tensor.matmul(
                                    out=pool_ps[h][:], lhsT=gih[:], rhs=x2b[:],
                                    start=(s == 0), stop=(s == n_slots - 1),
                                    skip_group_check=True)

                if not last:
                    nc.gpsimd.collective_compute(
                        "AllGather", mybir.AluOpType.bypass, replica_groups=rg,
                        ins=[T_sh[l + 1].ap().opt()],
                        outs=[T_full[l + 1].ap().opt()])

            # ---- head
            out_sb = spool.tile([P, G // P], F32, tag="outsb")
            for h in range(G // P):
                pool_sb = spool.tile([P, HID], F32, tag="poolsb",
                                     name=f"poolsb{h}")
                nc.vector.tensor_copy(out=pool_sb[:], in_=pool_ps[h][:])
                pt_ps = ppoolA.tile([HID, P], F32, tag="xtps")
                nc.tensor.transpose(out=pt_ps[:], in_=pool_sb[:],
                                    identity=ident_sb[:])
                pt_sb = spool.tile([HID, P], F32, tag="xtsb", name=f"ptsb{h}")
                nc.scalar.copy(out=pt_sb[:], in_=pt_ps[:])
                o_ps = ppoolA.tile([P, 1], F32, tag="tps", name=f"o_ps{h}")
                nc.tensor.matmul(out=o_ps[:], lhsT=pt_sb[:], rhs=wlin_sb[:],
                                 start=True, stop=True)
                nc.vector.tensor_copy(out=out_sb[:, h:h + 1], in_=o_ps[:])
            nc.sync.dma_start(out=out_d[:, :], in_=out_sb[:])

    nc.compile()
    return nc


# ----------------------------------------------------------------------------
# Entry point
# ----------------------------------------------------------------------------

def make_inmaps(pr: Prep, w: dict, T1, adcol1):
    identb = np.eye(P, dtype=np.float32).astype(BF)
    ident = np.eye(P, dtype=np.float32)
    iotg = np.tile(np.arange(G, dtype=np.float32)[None, :], (P, 1))
    in_maps = []
    for c in range(NCORES):
        m = {
            "T1": T1,
            "sidx": pr.sidx[c],
            "adcol1": adcol1[c],
            "gid": pr.gid[c],
            "winv": pr.winv[c],
            "wlin": w["wlin"],
            "identb": identb,
            "ident": ident,
            "iotg": iotg,
        }
        for li, l in enumerate((1, 2, 3)):
            m[f"attr{l}"] = pr.attr_m[li, c].astype(BF)
            m[f"brep{l}"] = w[f"brep{l}"]
        for l in (2, 3):
            m[f"wext{l}"] = w[f"wext{l}"]
        in_maps.append(m)
    return in_maps


def kernel(**inputs) -> np.ndarray:
    inputs = {k: np.asarray(v) for k, v in inputs.items()}
    w = pack_weights(inputs)
    pr = preprocess(inputs["edge_index"], inputs["edge_attr"], inputs["batch"],
                    [w["c1"], w["c2"], w["c3"]])
    T1, adcol1 = host_table1(pr, w, np.asarray(inputs["x"], np.float32))
    nc = build_program(pr)
    in_maps = make_inmaps(pr, w, T1, adcol1)
    res = bass_utils.run_bass_kernel_spmd(nc, in_maps,
                                          core_ids=list(range(NCORES)))
    out = np.zeros(G, np.float64)
    for c in range(NCORES):
        oc = res.results[c]["out"]
        out += oc.T.reshape(-1).astype(np.float64)
    return (out + w["blin"]).astype(np.float32)
